# revision 1
# baseline (speedup 1.0000x reference)
"""Trainium2 Bass kernel for nn_Decoder (mlp3 + segment_sum decoder), 8 cores.

Strategy (data-parallel over nodes N, segment-aligned shard boundaries):
  - Host: shard rows so core c owns whole segments [128c, 128(c+1)); transpose
    x to [H, rows] fp16 so stage-1 matmul lhsT streams straight from HBM;
    precompute folded biases (the ssp "-ln2" shift folded into the next
    stage's bias / the final projection's per-segment count correction).
  - Device (per core, SPMD): for each 128-row subtile, 5 passes (4 proc t's +
    enc) x 3 stages of: matmul (fp16, weights as moving operand), LayerNorm
    stats via bn_stats/bn_aggr, softplus as Exp (fused scale/bias = LN
    normalize) then Ln(x+1), transpose between stages via matmul-by-identity,
    and a one-hot segment matmul accumulating pooled sums in PSUM.
    rstd is computed as exp(-0.5*ln(var+eps)) to stay inside one ACT table set.
  - Final tiny projection (pooled @ wp/we + consts) on device; host gathers
    the per-core [128 segs, T] outputs into [T, G].
"""
import sys
sys.path.insert(0, "/opt/trn_rl_repo")
import numpy as np

T, N, H, G = 4, 100000, 128, 1024
NCORES = 8
SEG_PER_CORE = G // NCORES        # 128
LN2 = float(np.log(2.0))
EPS = 1e-5
P = 128
JG = 4                            # subtiles loaded per DMA group

_cache = {}


_tables_patched = False


def _patch_act_tables():
    """Make the table chooser put Exp and Ln in one set (one load total)."""
    global _tables_patched
    if _tables_patched:
        return
    import concourse.bacc as bacc
    from concourse import mybir, hw_specs
    _orig = hw_specs.get_activation_tables

    def patched(arch):
        t = _orig(arch)
        AF = mybir.ActivationFunctionType
        for name, funcs in t.items():
            if name != "natural_log_exp_and_others":
                t[name] = {f for f in funcs if f not in (AF.Exp, AF.Ln)}
        return t

    bacc.get_activation_tables = patched
    _tables_patched = True


def _build(nsub, use_bias0p, use_bias0e, route_b=None):
    import concourse.bass as bass
    import concourse.bacc as bacc
    import concourse.tile as tile
    from concourse import mybir
    _patch_act_tables()
    if route_b is None:
        route_b = lambda j, s: (s == 2) or (s == 1 and j % 2 == 0)
    F16, F32 = mybir.dt.float16, mybir.dt.float32
    AF = mybir.ActivationFunctionType
    OP = mybir.AluOpType

    BJ = 8                            # j's interleaved per pipeline block
    assert nsub % BJ == 0
    nloc = nsub * P
    nc = bacc.Bacc("TRN2", target_bir_lowering=False, debug=False,
                   enable_asserts=True, num_devices=NCORES)

    xt_proc = nc.dram_tensor("xt_proc", [T, H, nloc], F16, kind="ExternalInput").ap()
    xt_enc = nc.dram_tensor("xt_enc", [H, nloc], F16, kind="ExternalInput").ap()
    batch_loc = nc.dram_tensor("batch_loc", [P, nsub], F32, kind="ExternalInput").ap()
    pw16 = nc.dram_tensor("pw16", [3, H, H], F16, kind="ExternalInput").ap()
    ew16 = nc.dram_tensor("ew16", [3, H, H], F16, kind="ExternalInput").ap()
    pb16 = nc.dram_tensor("pb16", [3, H], F16, kind="ExternalInput").ap()
    eb16 = nc.dram_tensor("eb16", [3, H], F16, kind="ExternalInput").ap()
    ident = nc.dram_tensor("ident", [H, H], F16, kind="ExternalInput").ap()
    wp4b = nc.dram_tensor("wp4b", [P, T * H], F32, kind="ExternalInput").ap()
    web = nc.dram_tensor("web", [P, H], F32, kind="ExternalInput").ap()
    kvec = nc.dram_tensor("kvec", [P, 1], F32, kind="ExternalInput").ap()
    res = nc.dram_tensor("res", [P, T], F32, kind="ExternalOutput").ap()

    with tile.TileContext(nc) as tc:
        import contextlib
        with contextlib.ExitStack() as ctx:
            singles = ctx.enter_context(tc.tile_pool(name="singles", bufs=1))
            xload = ctx.enter_context(tc.tile_pool(name="xload", bufs=4))
            work = ctx.enter_context(tc.tile_pool(name="work", bufs=6))
            blockp = ctx.enter_context(tc.tile_pool(name="blockp", bufs=12))
            stat = ctx.enter_context(tc.tile_pool(name="stat", bufs=16))
            zpool = ctx.enter_context(tc.tile_pool(name="zp", bufs=3, space="PSUM"))
            etpool = ctx.enter_context(tc.tile_pool(name="etp", bufs=2, space="PSUM"))
            mis = ctx.enter_context(tc.tile_pool(name="mis", bufs=2, space="PSUM"))
            acc = ctx.enter_context(tc.tile_pool(name="acc", bufs=1, space="PSUM"))

            # --- one-time constants ---
            w16 = []   # [family][stage] -> [H, H] fp16 tile
            for fam, src in (("p", pw16), ("e", ew16)):
                fam_tiles = []
                for si in range(3):
                    wt = singles.tile([H, H], F16, tag=f"w{fam}{si}")
                    nc.sync.dma_start(out=wt, in_=src[si])
                    fam_tiles.append(wt)
                w16.append(fam_tiles)
            b16 = []
            for fam, src in (("p", pb16), ("e", eb16)):
                fam_tiles = []
                for si in range(3):
                    bt = singles.tile([1, H], F16, tag=f"b{fam}{si}")
                    nc.sync.dma_start(out=bt, in_=src[si:si + 1, :])
                    fam_tiles.append(bt)
                b16.append(fam_tiles)
            i16 = singles.tile([H, H], F16, tag="ident")
            nc.sync.dma_start(out=i16, in_=ident)
            ones16 = singles.tile([1, H], F16, tag="ones")
            nc.vector.memset(ones16, 1.0)
            eps_t = singles.tile([P, 1], F32, tag="eps")
            nc.vector.memset(eps_t, EPS)
            iota = singles.tile([P, P], F32, tag="iota")
            nc.gpsimd.iota(iota, pattern=[[1, P]], base=0, channel_multiplier=0,
                           allow_small_or_imprecise_dtypes=True)
            bl_all = singles.tile([P, nsub], F32, tag="bl_all")
            nc.sync.dma_start(out=bl_all, in_=batch_loc)
            wp4t = singles.tile([P, T * H], F32, tag="wp4")
            nc.sync.dma_start(out=wp4t, in_=wp4b)
            webt = singles.tile([P, H], F32, tag="web")
            nc.sync.dma_start(out=webt, in_=web)
            kvt = singles.tile([P, 1], F32, tag="kv")
            nc.sync.dma_start(out=kvt, in_=kvec)

            # --- pooled accumulators: proc in PSUM, enc in SBUF ---
            pp = acc.tile([P, T, H], F32, tag="pp")
            pe_sb = singles.tile([P, H], F32, tag="pe_sb")
            nc.vector.memset(pe_sb, 0.0)

            use_bias = [[use_bias0p, True, True], [use_bias0e, True, True]]

            for jb in range(nsub // BJ):
                jlist = list(range(jb * BJ, (jb + 1) * BJ))
                xg = {}
                for gi in range(BJ // JG):
                    jg = (jb * BJ) // JG + gi
                    for p in range(5):
                        xt = xload.tile([H, JG * P], F16, tag=f"x{p}_{gi}")
                        if p < T:
                            nc.sync.dma_start(
                                out=xt,
                                in_=xt_proc[p, :, jg * JG * P:(jg + 1) * JG * P])
                        else:
                            nc.sync.dma_start(
                                out=xt, in_=xt_enc[:, jg * JG * P:(jg + 1) * JG * P])
                        for jj in range(JG):
                            xg[(jg * JG + jj, p)] = xt[:, jj * P:(jj + 1) * P]

                s16_by_j = {}
                for j in jlist:
                    s16 = blockp.tile([P, P], F16, tag="s16")
                    nc.vector.tensor_scalar(out=s16, in0=iota,
                                            scalar1=bl_all[:, j:j + 1], scalar2=0.0,
                                            op0=OP.is_equal, op1=OP.bypass)
                    s16_by_j[j] = s16

                lhs_by_j = {j: [xg[(j, p)] for p in range(5)] for j in jlist}

                for s in range(3):
                    def emit_A(j):
                        lhs = lhs_by_j[j]
                        z4 = zpool.tile([P, T * H], F32, tag="z4")
                        zm = mis.tile([P, 4, H], F32, tag="zm")
                        # column-interleaved pairs: proc subtile p lives at
                        # stride-2 columns (pair g=p//2, parity p%2), so one
                        # bn_stats over the contiguous pair region yields both
                        # subtiles' stats via the even/odd split.
                        def zsl(p):
                            if p == 4:
                                return zm[:, 0, :]
                            g, par = divmod(p, 2)
                            return bass.AP(tensor=z4.tensor,
                                           offset=z4.offset + g * 2 * H + par,
                                           ap=[z4.ap[0], [2, H]])
                        zs = [zsl(p) for p in range(5)]
                        for p in range(5):
                            fam = 0 if p < T else 1
                            has_b = use_bias[fam][s]
                            # pair-mate mms share one zero-region: only the
                            # even-parity mm starts the group (per-element
                            # has_written makes the odd overwrite correct)
                            st_flag = (p % 2 == 0) or (p == 4)
                            sp_flag = ((p % 2 == 1) or (p == 4)) and not has_b
                            nc.tensor.matmul(zs[p], lhsT=lhs[p], rhs=w16[fam][s],
                                             start=st_flag, stop=sp_flag)
                            if has_b:
                                nc.tensor.matmul(zs[p], lhsT=ones16, rhs=b16[fam][s],
                                                 start=False,
                                                 stop=(p % 2 == 1) or (p == 4))
                        # stc: [enc mv(2) | pair0 st(6) | pair1 st(6)]
                        #  -> means at {0,3,6,9,12}, M2s at {1,4,7,10,13}
                        stc = stat.tile([P, 14], F32, tag="stc")
                        nc.vector.bn_stats(out=stc[:, 2:8], in_=z4[:, 0:2 * H])
                        nc.vector.bn_stats(out=stc[:, 8:14], in_=z4[:, 2 * H:4 * H])
                        st_enc = stat.tile([P, 6], F32, tag="st_enc")
                        nc.vector.bn_stats(out=st_enc, in_=zm[:, 0, :])
                        nc.vector.bn_aggr(out=stc[:, 0:2], in_=st_enc)
                        # bring enc var to raw-M2 scale (M2 = n * var)
                        nc.vector.tensor_scalar(out=stc[:, 1:2], in0=stc[:, 1:2],
                                                scalar1=float(P), scalar2=0.0,
                                                op0=OP.mult, op1=OP.bypass)
                        means5 = bass.AP(tensor=stc.tensor, offset=stc.offset,
                                         ap=[stc.ap[0], [3, 5]])
                        m2s5 = bass.AP(tensor=stc.tensor, offset=stc.offset + 1,
                                       ap=[stc.ap[0], [3, 5]])
                        # rstd = exp(-0.5 * ln(M2/128 + eps)) (one ACT table set)
                        l5 = stat.tile([P, 5], F32, tag="l5")
                        nc.scalar.activation(out=l5, in_=m2s5, func=AF.Ln,
                                             bias=eps_t, scale=1.0 / P)
                        r5 = stat.tile([P, 5], F32, tag="r5")
                        nc.scalar.activation(out=r5, in_=l5, func=AF.Exp,
                                             bias=0.0, scale=-0.5)
                        return zs, zm, means5, r5

                    def emit_B(j, state):
                        zs, zm, means5, r5 = state
                        e5 = work.tile([P, 5, H], F16, tag="e5")
                        if route_b(j, s):
                            zn5 = work.tile([P, 5, H], F32, tag="zn5")
                            for p in range(5):
                                nc.vector.tensor_scalar(
                                    out=zn5[:, p, :], in0=zs[p],
                                    scalar1=r5[:, (p + 1) % 5:(p + 1) % 5 + 1],
                                    scalar2=0.0,
                                    op0=OP.mult, op1=OP.bypass)
                            nc.scalar.activation(out=e5, in_=zn5, func=AF.Exp,
                                                 bias=0.0, scale=1.0)
                        else:
                            for p in range(5):
                                nc.scalar.activation(out=e5[:, p, :], in_=zs[p],
                                                     func=AF.Exp, bias=0.0,
                                                     scale=r5[:, (p + 1) % 5:(p + 1) % 5 + 1])
                        if s < 2:
                            et4 = etpool.tile([P, T, H], F32, tag="et4")
                            ets = [et4[:, t, :] for t in range(T)] + [zm[:, 1, :]]
                            for p in range(5):
                                nc.tensor.matmul(ets[p], lhsT=e5[:, p, :], rhs=i16,
                                                 start=True, stop=True)
                            at5 = blockp.tile([P, 5, H], F16, tag="at5")
                            nc.scalar.activation(out=at5[:, 0:T, :], in_=et4,
                                                 func=AF.Ln, bias=1.0, scale=1.0)
                            nc.scalar.activation(out=at5[:, T, :], in_=ets[T],
                                                 func=AF.Ln, bias=1.0, scale=1.0)
                            lhs_by_j[j] = [at5[:, p, :] for p in range(5)]
                        else:
                            a5 = work.tile([P, 5, H], F16, tag="a5")
                            nc.scalar.activation(out=a5, in_=e5, func=AF.Ln,
                                                 bias=1.0, scale=1.0)
                            for p in range(T):
                                nc.tensor.matmul(pp[:, p, :], lhsT=s16_by_j[j],
                                                 rhs=a5[:, p, :],
                                                 start=(j == 0 and p == 0),
                                                 stop=(j == nsub - 1 and p == T - 1))
                            # enc pooled partial -> spare zm slot -> SBUF acc
                            nc.tensor.matmul(zm[:, 2, :], lhsT=s16_by_j[j],
                                             rhs=a5[:, T, :], start=True, stop=True)
                            nc.vector.tensor_tensor(out=pe_sb, in0=pe_sb,
                                                    in1=zm[:, 2, :], op=OP.add)

                    # skewed software pipeline, depth 2: A runs two j's
                    # ahead of B so cross-engine chains overlap
                    from collections import deque
                    pend = deque()
                    for j in jlist:
                        pend.append((j, emit_A(j)))
                        if len(pend) > 2:
                            jj_, st_ = pend.popleft()
                            emit_B(jj_, st_)
                    while pend:
                        jj_, st_ = pend.popleft()
                        emit_B(jj_, st_)

            # --- final projection ---
            ppf = singles.tile([P, T, H], F32, tag="ppf")
            nc.vector.tensor_copy(out=ppf, in_=pp)
            pef = pe_sb
            ppm = singles.tile([P, T * H], F32, tag="ppm")
            nc.vector.tensor_tensor(out=ppm, in0=ppf.rearrange("p a b -> p (a b)"),
                                    in1=wp4t, op=OP.mult)
            pem = singles.tile([P, H], F32, tag="pem")
            nc.vector.tensor_tensor(out=pem, in0=pef, in1=webt, op=OP.mult)
            projp = singles.tile([P, T], F32, tag="projp")
            nc.vector.reduce_sum(out=projp, in_=ppm.rearrange("p (a b) -> p a b", a=T),
                                 axis=mybir.AxisListType.X)
            proje = singles.tile([P, 1], F32, tag="proje")
            nc.vector.reduce_sum(out=proje, in_=pem, axis=mybir.AxisListType.X)
            rest = singles.tile([P, T], F32, tag="rest")
            nc.vector.tensor_scalar(out=rest, in0=projp, scalar1=proje, scalar2=kvt,
                                    op0=OP.add, op1=OP.add)
            nc.sync.dma_start(out=res, in_=rest)

    nc.compile()
    return nc


class _Runner:
    """Holds the jitted PJRT callable for repeated execution."""

    def __init__(self, nc, n_cores):
        import jax
        from jax.sharding import Mesh, PartitionSpec
        from jax.experimental.shard_map import shard_map
        from concourse import mybir
        from concourse.bass2jax import (_bass_exec_p, install_neuronx_cc_hook,
                                        partition_id_tensor)
        install_neuronx_cc_hook()
        self.jax = jax
        self.n_cores = n_cores
        partition_name = nc.partition_id_tensor.name if nc.partition_id_tensor else None
        dbg_name = nc.dbg_addr.name if nc.dbg_addr else None
        in_names, out_names, out_avals, zero_outs = [], [], [], []
        for alloc in nc.m.functions[0].allocations:
            if not isinstance(alloc, mybir.MemoryLocationSet):
                continue
            name = alloc.memorylocations[0].name
            if alloc.kind == "ExternalInput":
                if name not in (partition_name, dbg_name):
                    in_names.append(name)
            elif alloc.kind == "ExternalOutput":
                shape = tuple(alloc.tensor_shape)
                dtype = mybir.dt.np(alloc.dtype)
                out_names.append(name)
                out_avals.append(jax.core.ShapedArray(shape, dtype))
                zero_outs.append(np.zeros(shape, dtype))
        self.in_names, self.out_names = in_names, out_names
        self.out_avals, self.zero_outs = out_avals, zero_outs
        all_in = list(in_names) + list(out_names)
        if dbg_name is not None:
            all_in.append(dbg_name)
        if partition_name is not None:
            all_in.append(partition_name)

        def _body(*args):
            operands = list(args)
            if dbg_name is not None:
                operands.append(jax.numpy.zeros((1, 2), jax.numpy.uint32))
            if partition_name is not None:
                operands.append(partition_id_tensor())
            return tuple(_bass_exec_p.bind(
                *operands, out_avals=tuple(out_avals), in_names=tuple(all_in),
                out_names=tuple(out_names), lowering_input_output_aliases=(),
                sim_require_finite=True, sim_require_nnan=True, nc=nc))

        devices = jax.devices()[:n_cores]
        self.mesh = Mesh(np.asarray(devices), ("core",))
        n_io = len(in_names) + len(out_names)
        self.fn = jax.jit(
            shard_map(_body, mesh=self.mesh,
                      in_specs=(PartitionSpec("core"),) * n_io,
                      out_specs=(PartitionSpec("core"),) * len(out_names),
                      check_rep=False),
            keep_unused=True)

    def prepare(self, in_maps):
        import jax
        from jax.sharding import PartitionSpec
        n = self.n_cores
        sharding = jax.sharding.NamedSharding(self.mesh, PartitionSpec("core"))
        dev_in = [jax.device_put(
            np.concatenate([np.asarray(in_maps[c][name]) for c in range(n)], axis=0),
            sharding) for name in self.in_names]
        dev_zero = [jax.device_put(
            np.zeros((n * z.shape[0], *z.shape[1:]), z.dtype), sharding)
            for z in self.zero_outs]
        return dev_in, dev_zero

    def run(self, handle):
        dev_in, dev_zero = handle
        outs = self.fn(*dev_in, *dev_zero)
        self.jax.block_until_ready(outs)
        return outs

    def results(self, outs):
        n = self.n_cores
        return [{name: np.asarray(outs[i]).reshape(n, *self.out_avals[i].shape)[c]
                 for i, name in enumerate(self.out_names)} for c in range(n)]


def _prep_inputs(x_proc, x_enc, batch, pW, pb, pg, pbt, eW, eb, eg, ebt,
                 wp, bp, we, be):
    """Host-side sharding + precomputation. Returns (in_maps, meta)."""
    x_proc = np.asarray(x_proc, dtype=np.float32)
    x_enc = np.asarray(x_enc, dtype=np.float32)
    batch = np.asarray(batch).astype(np.int64)
    pW = np.asarray(pW, dtype=np.float32)
    eW = np.asarray(eW, dtype=np.float32)
    pb = np.asarray(pb, dtype=np.float32)
    eb = np.asarray(eb, dtype=np.float32)
    wp = np.asarray(wp, dtype=np.float32).reshape(H)
    we = np.asarray(we, dtype=np.float32).reshape(H)
    bp = float(np.asarray(bp).reshape(-1)[0])
    be = float(np.asarray(be).reshape(-1)[0])

    assert np.allclose(np.asarray(pg), 1) and np.allclose(np.asarray(eg), 1), \
        "kernel assumes LN gain == 1"
    assert np.allclose(np.asarray(pbt), 0) and np.allclose(np.asarray(ebt), 0), \
        "kernel assumes LN shift == 0"

    splits = np.searchsorted(batch, np.arange(NCORES + 1) * SEG_PER_CORE)
    rows = splits[1:] - splits[:-1]
    nloc_raw = int(rows.max())
    nsub = max(1, -(-nloc_raw // P))
    nsub = -(-nsub // 8) * 8            # multiple of the pipeline block
    nloc = nsub * P

    def center16(W):
        # fold LN's mean subtraction into the weights: one rounding only --
        # the post-rounding row-mean residual (~1e-6) is already negligible
        return (W - W.mean(axis=-1, keepdims=True)).astype(np.float16)

    pw16 = np.stack([center16(pW[i]) for i in range(3)])
    ew16 = np.stack([center16(eW[i]) for i in range(3)])

    def beff(b, W16):
        e = np.stack([b[0],
                      b[1] - LN2 * W16[1].astype(np.float32).sum(0),
                      b[2] - LN2 * W16[2].astype(np.float32).sum(0)])
        return e - e.mean(axis=-1, keepdims=True)

    pb_eff = beff(pb, pw16)
    eb_eff = beff(eb, ew16)
    use_bias0p = bool(np.abs(pb_eff[0]).max() > 1e-7)
    use_bias0e = bool(np.abs(eb_eff[0]).max() > 1e-7)

    ident = np.eye(H, dtype=np.float16)
    wp4b = np.tile(wp[None, :], (P, T)).astype(np.float32)        # [P, T*H]
    web = np.tile(we[None, :], (P, 1)).astype(np.float32)         # [P, H]

    in_maps = []
    for c in range(NCORES):
        lo, hi = int(splits[c]), int(splits[c + 1])
        n_c = hi - lo
        xtp = np.zeros((T, H, nloc), np.float16)
        xtp[:, :, :n_c] = x_proc[:, lo:hi, :].transpose(0, 2, 1).astype(np.float16)
        xte = np.zeros((H, nloc), np.float16)
        xte[:, :n_c] = x_enc[lo:hi, :].T.astype(np.float16)
        bl = np.full(nloc, -1.0, np.float32)
        bl[:n_c] = (batch[lo:hi] - c * SEG_PER_CORE).astype(np.float32)
        cnt = np.zeros(SEG_PER_CORE, np.float64)
        segs, counts = np.unique(batch[lo:hi], return_counts=True)
        cnt[(segs - c * SEG_PER_CORE).astype(int)] = counts
        kv = (bp + be - LN2 * cnt * (wp.sum() + we.sum())).astype(np.float32)
        in_maps.append({
            "xt_proc": xtp, "xt_enc": xte,
            # [P, nsub]: column j = the 128 batch ids of subtile j
            "batch_loc": bl.reshape(nsub, P).T.copy(),
            "pw16": pw16, "ew16": ew16,
            "pb16": pb_eff.astype(np.float16), "eb16": eb_eff.astype(np.float16),
            "ident": ident, "wp4b": wp4b, "web": web,
            "kvec": kv.reshape(P, 1),
        })
    meta = (nsub, use_bias0p, use_bias0e)
    return in_maps, meta


def get_runner(meta):
    key = meta
    if key not in _cache:
        nc = _build(*meta)
        _cache[key] = _Runner(nc, NCORES)
    return _cache[key]


def kernel(**inputs) -> np.ndarray:
    in_maps, meta = _prep_inputs(**inputs)
    runner = get_runner(meta)
    handle = runner.prepare(in_maps)
    outs = runner.run(handle)
    per_core = runner.results(outs)
    out = np.zeros((T, G), np.float32)
    for c in range(NCORES):
        out[:, c * SEG_PER_CORE:(c + 1) * SEG_PER_CORE] = per_core[c]["res"].T
    return out

